# revision 2
# baseline (speedup 1.0000x reference)
"""Chamfer distance (B=8, N=M=4096, D=3) on 8 Trainium2 NeuronCores.

Strategy: data-parallel over batch -- core b computes batch element b.

On this execution path the dominant cost is ~25-60us PER UNIQUE PROGRAM
INSTRUCTION per execution (program-size bound, not compute bound), while
instructions executed inside hardware loops run at real engine speed
(~us) with sub-us back-edges. The kernel is therefore built raw (no
TileContext) as three small independent per-engine hardware loops
(Act / PE / DVE) synchronized by semaphores whose wait thresholds are
each engine's own loop register -- no all-engine barriers, no back-edge
drains, ~70 total instructions (the TileContext equivalent is ~170, the
original unrolled kernel ~1150).

Math: squared distances via the Gram trick, computed NEGATED on the
tensor engine with homogeneous coordinates in bf16, 3-way hi/mid/lo
splitting (products kept for split orders i+j<=2):
  sum_k S_A[k,n] * S_B[k,m] = 2<x,y> - |x|^2 - |y|^2 = -d2[n,m]
S = [S_A | S_B] : [24, 8192] bf16, built host-side (bit twiddling).

Both Chamfer directions become row-min reductions (no partition-axis
reduction anywhere):
  pass1: x-block i (stationary) vs all y (moving) -> min over m
  pass2: y-block i (stationary) vs all x (moving) -> min over n

Per iteration i (32 per kernel block):
  Act : st1 <- S[:, i*128:+128]   (waits m1b>=i: PE done with st1)
        st2 <- S[:, N+i*128:+128] (waits m2b>=i)        -> sa1/sa2
  PE  : G1a 4mm ps1 (waits sa1>=i+1, r1b>=i)            -> m1a
        G1b 4mm ps1 (waits r1a>=i+1)                    -> m1b
        G2a 4mm ps2 (waits sa2>=i+1, r2b>=i)            -> m2a
        G2b 4mm ps2 (waits r2a>=i+1)                    -> m2b
  DVE : reduce-max ps1 -> cm[:, i]       (waits m1a>=i+1) -> r1a
        reduce-max ps1 -> cm[:, 2NBR+i]  (waits m1b>=i+1) -> r1b
        reduce-max ps2 -> cm[:, NBR+i]   (waits m2a>=i+1) -> r2a
        reduce-max ps2 -> cm[:, 3NBR+i]  (waits m2b>=i+1) -> r2b
Tail (DVE): cm[:,0:2NBR] = max(halves); relu(-x); row-sum -> [128,1].
SP: input DMA (then_inc; Act pre-waits), output DMA after tail sem.
Host sums the per-core [128,1] partials, / (B*N).

`repeat` duplicates the whole loop-triplet block (distinct instructions,
continuing semaphore thresholds) so a repeat-delta wall-clock measures
the true per-kernel-block cost. repeat=1 is the graded program.
"""

import os
import sys

import numpy as np

for _p in ("/opt/trn_rl_repo", "/root/.axon_site/_ro/trn_rl_repo"):
    if os.path.isdir(_p) and _p not in sys.path:
        sys.path.append(_p)

B, N, M, D = 8, 4096, 4096, 3
P = 128
NCORES = 8
KA = 24            # augmented contraction rows (3-way bf16 split)
NB = N // P        # 32 n-blocks
HW_ = 2048         # half of m per psum tile (4 banks)

_PROG = None


def _build_program(repeat: int = 1):
    import concourse.mybir as mybir
    from concourse import bacc
    from concourse.bass import ds, ts

    f32 = mybir.dt.float32
    bf16 = mybir.dt.bfloat16
    u16 = mybir.dt.uint16
    Alu = mybir.AluOpType
    Ax = mybir.AxisListType

    nc = bacc.Bacc("TRN2", target_bir_lowering=False, debug=False,
                   num_devices=NCORES)
    sd = nc.dram_tensor("s", [KA, N + M], u16, kind="ExternalInput").ap()
    outd = nc.dram_tensor("out", [P, 1], f32, kind="ExternalOutput").ap()

    NBR = NB * repeat

    # pad S so repeat>1 timing builds keep dynamic stage APs in range
    # (the pad is uninitialized, read only when repeat > 1)
    S = nc.alloc_sbuf_tensor("S", [KA, N + M + (NBR - NB) * P], bf16).ap()
    st1 = nc.alloc_sbuf_tensor("st1", [KA, P], bf16).ap()
    st2 = nc.alloc_sbuf_tensor("st2", [KA, P], bf16).ap()
    cm = nc.alloc_sbuf_tensor("cm", [P, 4 * NBR], f32).ap()
    neg = nc.alloc_sbuf_tensor("neg", [P, 2 * NBR], f32).ap()
    outsb = nc.alloc_sbuf_tensor("outsb", [P, 1], f32).ap()
    ps1 = nc.alloc_psum_tensor("ps1", [P, HW_], f32).ap()
    ps2 = nc.alloc_psum_tensor("ps2", [P, HW_], f32).ap()

    sdma = nc.alloc_semaphore("sdma")
    sa1 = nc.alloc_semaphore("sa1")
    sa2 = nc.alloc_semaphore("sa2")
    m1a = nc.alloc_semaphore("m1a")
    m1b = nc.alloc_semaphore("m1b")
    m2a = nc.alloc_semaphore("m2a")
    m2b = nc.alloc_semaphore("m2b")
    r1a = nc.alloc_semaphore("r1a")
    r1b = nc.alloc_semaphore("r1b")
    r2a = nc.alloc_semaphore("r2a")
    r2b = nc.alloc_semaphore("r2b")
    stail = nc.alloc_semaphore("stail")
    ttail = nc.alloc_semaphore("ttail")
    sdone = nc.alloc_semaphore("sdone")

    # ---- SP: input DMA ----
    nc.sync.dma_start(S[:, 0:N + M].bitcast(u16), sd).then_inc(sdma, 16)

    nc.scalar.wait_ge(sdma, 16)
    for d in range(repeat):
        lo, hi = d * NB, (d + 1) * NB

        # ---- Act loop: stage copies ----
        with nc.scalar.Fori(lo, hi, 1) as i:
            nc.scalar.wait_ge(m1b, i)
            nc.scalar.copy(st1, S[:, ts(i, P)]).then_inc(sa1)
            nc.scalar.wait_ge(m2b, i)
            nc.scalar.copy(st2, S[:, ds(i * P + N, P)]).then_inc(sa2)

        # ---- PE loop: 16 self-loading matmuls ----
        with nc.tensor.Fori(lo, hi, 1) as i:
            nc.tensor.wait_ge(sa1, i + 1)
            nc.tensor.wait_ge(r1b, i)
            for j in range(4):
                mm = nc.tensor.matmul(
                    ps1[:, 512 * j:512 * (j + 1)], lhsT=st1,
                    rhs=S[:, N + 512 * j:N + 512 * (j + 1)],
                    start=True, stop=True)
            mm.then_inc(m1a)
            nc.tensor.wait_ge(r1a, i + 1)
            for j in range(4):
                mm = nc.tensor.matmul(
                    ps1[:, 512 * j:512 * (j + 1)], lhsT=st1,
                    rhs=S[:, N + HW_ + 512 * j:N + HW_ + 512 * (j + 1)],
                    start=True, stop=True)
            mm.then_inc(m1b)
            nc.tensor.wait_ge(sa2, i + 1)
            nc.tensor.wait_ge(r2b, i)
            for j in range(4):
                mm = nc.tensor.matmul(
                    ps2[:, 512 * j:512 * (j + 1)], lhsT=st2,
                    rhs=S[:, 512 * j:512 * (j + 1)],
                    start=True, stop=True)
            mm.then_inc(m2a)
            nc.tensor.wait_ge(r2a, i + 1)
            for j in range(4):
                mm = nc.tensor.matmul(
                    ps2[:, 512 * j:512 * (j + 1)], lhsT=st2,
                    rhs=S[:, HW_ + 512 * j:HW_ + 512 * (j + 1)],
                    start=True, stop=True)
            mm.then_inc(m2b)

        # ---- DVE loop: 4 row-max reductions ----
        with nc.vector.Fori(lo, hi, 1) as i:
            nc.vector.wait_ge(m1a, i + 1)
            nc.vector.tensor_reduce(
                cm[:, ds(i, 1)], ps1, axis=Ax.X, op=Alu.max).then_inc(r1a)
            nc.vector.wait_ge(m1b, i + 1)
            nc.vector.tensor_reduce(
                cm[:, ds(i + 2 * NBR, 1)], ps1, axis=Ax.X,
                op=Alu.max).then_inc(r1b)
            nc.vector.wait_ge(m2a, i + 1)
            nc.vector.tensor_reduce(
                cm[:, ds(i + NBR, 1)], ps2, axis=Ax.X,
                op=Alu.max).then_inc(r2a)
            nc.vector.wait_ge(m2b, i + 1)
            nc.vector.tensor_reduce(
                cm[:, ds(i + 3 * NBR, 1)], ps2, axis=Ax.X,
                op=Alu.max).then_inc(r2b)

    # ---- tail on DVE ----
    # same-queue waits: no-ops at runtime, explicit sync edges for the
    # race detector against the loop's dynamic-AP reduce writes
    nc.vector.wait_ge(r1a, NBR)
    nc.vector.wait_ge(r1b, NBR)
    nc.vector.wait_ge(r2a, NBR)
    nc.vector.wait_ge(r2b, NBR)
    nc.vector.tensor_max(cm[:, 0:2 * NBR], cm[:, 0:2 * NBR],
                         cm[:, 2 * NBR:4 * NBR]).then_inc(ttail)
    nc.vector.wait_ge(ttail, 1)
    nc.vector.tensor_scalar(
        out=neg, in0=cm[:, 0:2 * NBR], scalar1=-1.0, scalar2=0.0,
        op0=Alu.mult, op1=Alu.max).then_inc(ttail)
    nc.vector.wait_ge(ttail, 2)
    nc.vector.tensor_reduce(
        outsb, neg, axis=Ax.X, op=Alu.add).then_inc(stail)

    # ---- SP: output DMA ----
    nc.sync.wait_ge(stail, 1)
    nc.sync.dma_start(outd, outsb).then_inc(sdone, 16)
    nc.sync.wait_ge(sdone, 16)

    nc.compile()
    return nc


def _get_program():
    global _PROG
    if _PROG is None:
        _PROG = _build_program()
    return _PROG


def _bf16_round(v: np.ndarray) -> np.ndarray:
    """Round f32 -> nearest-even bf16, returned as f32 with low bits zero."""
    u = np.ascontiguousarray(v, dtype=np.float32).view(np.uint32)
    r = ((u >> 16) & 1) + np.uint32(0x7FFF)
    return ((u + r) & np.uint32(0xFFFF0000)).view(np.float32)


# (i, j) split-product pairs kept: i + j <= 2
_PAIRS = [(0, 0), (0, 1), (0, 2), (1, 0), (1, 1), (2, 0)]


def _split3(v: np.ndarray):
    v = v.astype(np.float32)
    s0 = _bf16_round(v)
    s1 = _bf16_round(v - s0)
    s2 = _bf16_round(v - s0 - s1)
    return s0, s1, s2


def _prep_sides(x: np.ndarray, y: np.ndarray):
    """Build augmented [KA, N] / [KA, M] bf16 matrices (as uint16 bits).

    sum_k A[k,n] * Bm[k,m] = 2<x_n, y_m> - |x_n|^2 - |y_m|^2 = -d2[n,m]
    """
    xs = _split3(x)                      # x ~ xs0+xs1+xs2
    cs = _split3(2.0 * y)                # 2y ~ cs0+cs1+cs2
    nx = _split3((x.astype(np.float64) ** 2).sum(-1).astype(np.float32))
    nyn = _split3(-(y.astype(np.float64) ** 2).sum(-1).astype(np.float32))

    A = np.empty((KA, x.shape[0]), np.float32)
    Bm = np.empty((KA, y.shape[0]), np.float32)
    r = 0
    for d in range(D):
        for (ii, jj) in _PAIRS:
            A[r] = xs[ii][:, d]
            Bm[r] = cs[jj][:, d]
            r += 1
    for k in range(3):
        A[r] = nx[k]
        Bm[r] = -1.0
        r += 1
    for k in range(3):
        A[r] = 1.0
        Bm[r] = nyn[k]
        r += 1
    assert r == KA
    Au = (np.ascontiguousarray(A).view(np.uint32) >> 16).astype(np.uint16)
    Bu = (np.ascontiguousarray(Bm).view(np.uint32) >> 16).astype(np.uint16)
    return Au, Bu


def _prep_s(x: np.ndarray, y: np.ndarray) -> np.ndarray:
    """Augmented matrix pair packed as one [KA, N+M] u16 array."""
    Au, Bu = _prep_sides(x, y)
    return np.concatenate([Au, Bu], axis=1)


def kernel(xyz1: np.ndarray, xyz2: np.ndarray) -> np.ndarray:
    from concourse.bass_utils import run_bass_kernel_spmd

    xyz1 = np.asarray(xyz1, dtype=np.float32)
    xyz2 = np.asarray(xyz2, dtype=np.float32)
    assert xyz1.shape == (B, N, D) and xyz2.shape == (B, M, D)

    nc = _get_program()
    in_maps = []
    for b in range(NCORES):
        in_maps.append({"s": _prep_s(xyz1[b], xyz2[b])})
    res = run_bass_kernel_spmd(nc, in_maps, list(range(NCORES))).results
    total = 0.0
    for r in res:
        total += float(r["out"].astype(np.float64).sum())
    # mean(dist1) + mean(dist2) = (sum dist1 + sum dist2) / (B*N)  (N == M)
    return np.float32(total / (B * N))


# revision 3
# speedup vs baseline: 2.3792x; 2.3792x over previous
"""Chamfer distance (B=8, N=M=4096, D=3) on 8 Trainium2 NeuronCores.

Strategy: data-parallel over batch -- core b computes batch element b.

On this execution path the dominant cost is ~25-60us PER UNIQUE PROGRAM
INSTRUCTION per execution (program-size bound, not compute bound), while
instructions executed inside hardware loops run at real engine speed
(~us) with sub-us back-edges. The kernel is therefore built raw (no
TileContext) as three small independent per-engine hardware loops
(Act / PE / DVE) synchronized by semaphores whose wait thresholds are
each engine's own loop register -- no all-engine barriers, no back-edge
drains, ~70 total instructions (the TileContext equivalent is ~170, the
original unrolled kernel ~1150).

Math: squared distances via the Gram trick, computed NEGATED on the
tensor engine with homogeneous coordinates in bf16, 3-way hi/mid/lo
splitting (products kept for split orders i+j<=2):
  sum_k S_A[k,n] * S_B[k,m] = 2<x,y> - |x|^2 - |y|^2 = -d2[n,m]
S = [S_A | S_B] : [24, 8192] bf16, built host-side (bit twiddling).

Both Chamfer directions become row-min reductions (no partition-axis
reduction anywhere):
  pass1: x-block i (stationary) vs all y (moving) -> min over m
  pass2: y-block i (stationary) vs all x (moving) -> min over n

Per iteration i (32 per kernel block):
  Act : st1 <- S[:, i*128:+128]   (waits m1b>=i: PE done with st1)
        st2 <- S[:, N+i*128:+128] (waits m2b>=i)        -> sa1/sa2
  PE  : G1a 4mm ps1 (waits sa1>=i+1, r1b>=i)            -> m1a
        G1b 4mm ps1 (waits r1a>=i+1)                    -> m1b
        G2a 4mm ps2 (waits sa2>=i+1, r2b>=i)            -> m2a
        G2b 4mm ps2 (waits r2a>=i+1)                    -> m2b
  DVE : reduce-max ps1 -> cm[:, i]       (waits m1a>=i+1) -> r1a
        reduce-max ps1 -> cm[:, 2NBR+i]  (waits m1b>=i+1) -> r1b
        reduce-max ps2 -> cm[:, NBR+i]   (waits m2a>=i+1) -> r2a
        reduce-max ps2 -> cm[:, 3NBR+i]  (waits m2b>=i+1) -> r2b
Tail (DVE): cm[:,0:2NBR] = max(halves); relu(-x); row-sum -> [128,1].
SP: input DMA (then_inc; Act pre-waits), output DMA after tail sem.
Host sums the per-core [128,1] partials, / (B*N).

`repeat` duplicates the whole loop-triplet block (distinct instructions,
continuing semaphore thresholds) so a repeat-delta wall-clock measures
the true per-kernel-block cost. repeat=1 is the graded program.
"""

import os
import sys

import numpy as np

for _p in ("/opt/trn_rl_repo", "/root/.axon_site/_ro/trn_rl_repo"):
    if os.path.isdir(_p) and _p not in sys.path:
        sys.path.append(_p)

B, N, M, D = 8, 4096, 4096, 3
P = 128
NCORES = 8
KA = 24            # augmented contraction rows (3-way bf16 split)
NB = N // P        # 32 n-blocks
HW_ = 2048         # half of m per psum tile (4 banks)

_PROG = None


def _build_program(repeat: int = 1):
    import concourse.mybir as mybir
    from concourse import bacc
    from concourse.bass import ds, ts

    f32 = mybir.dt.float32
    bf16 = mybir.dt.bfloat16
    u16 = mybir.dt.uint16
    Alu = mybir.AluOpType
    Ax = mybir.AxisListType

    nc = bacc.Bacc("TRN2", target_bir_lowering=False, debug=False,
                   num_devices=NCORES)
    sd = nc.dram_tensor("s", [KA, N + M], u16, kind="ExternalInput").ap()
    outd = nc.dram_tensor("out", [P, 1], f32, kind="ExternalOutput").ap()

    NBR = NB * repeat

    # pad S so repeat>1 timing builds keep dynamic stage APs in range
    # (the pad is uninitialized, read only when repeat > 1)
    S = nc.alloc_sbuf_tensor("S", [KA, N + M + (NBR - NB) * P], bf16).ap()
    st1 = nc.alloc_sbuf_tensor("st1", [KA, P], bf16).ap()
    st2 = nc.alloc_sbuf_tensor("st2", [KA, P], bf16).ap()
    cm = nc.alloc_sbuf_tensor("cm", [P, 4 * NBR], f32).ap()
    neg = nc.alloc_sbuf_tensor("neg", [P, 2 * NBR], f32).ap()
    outsb = nc.alloc_sbuf_tensor("outsb", [P, 1], f32).ap()
    ps1 = nc.alloc_psum_tensor("ps1", [P, HW_], f32).ap()
    ps2 = nc.alloc_psum_tensor("ps2", [P, HW_], f32).ap()

    sdma = nc.alloc_semaphore("sdma")
    sa1 = nc.alloc_semaphore("sa1")
    sa2 = nc.alloc_semaphore("sa2")
    m1a = nc.alloc_semaphore("m1a")
    m1b = nc.alloc_semaphore("m1b")
    m2a = nc.alloc_semaphore("m2a")
    m2b = nc.alloc_semaphore("m2b")
    r1a = nc.alloc_semaphore("r1a")
    r1b = nc.alloc_semaphore("r1b")
    r2a = nc.alloc_semaphore("r2a")
    r2b = nc.alloc_semaphore("r2b")
    stail = nc.alloc_semaphore("stail")
    ttail = nc.alloc_semaphore("ttail")
    sdone = nc.alloc_semaphore("sdone")

    # ---- SP: input DMA ----
    nc.sync.dma_start(S[:, 0:N + M].bitcast(u16), sd).then_inc(sdma, 16)

    nc.scalar.wait_ge(sdma, 16)
    for d in range(repeat):
        lo, hi = d * NB, (d + 1) * NB

        # ---- Act loop: stage copies ----
        with nc.scalar.Fori(lo, hi, 1) as i:
            nc.scalar.wait_ge(m1b, i)
            nc.scalar.copy(st1, S[:, ts(i, P)]).then_inc(sa1)
            nc.scalar.wait_ge(m2b, i)
            nc.scalar.copy(st2, S[:, ds(i * P + N, P)]).then_inc(sa2)

        # ---- PE loop: 16 self-loading matmuls ----
        with nc.tensor.Fori(lo, hi, 1) as i:
            nc.tensor.wait_ge(sa1, i + 1)
            nc.tensor.wait_ge(r1b, i)
            for j in range(4):
                mm = nc.tensor.matmul(
                    ps1[:, 512 * j:512 * (j + 1)], lhsT=st1,
                    rhs=S[:, N + 512 * j:N + 512 * (j + 1)],
                    start=True, stop=True)
            mm.then_inc(m1a)
            nc.tensor.wait_ge(r1a, i + 1)
            for j in range(4):
                mm = nc.tensor.matmul(
                    ps1[:, 512 * j:512 * (j + 1)], lhsT=st1,
                    rhs=S[:, N + HW_ + 512 * j:N + HW_ + 512 * (j + 1)],
                    start=True, stop=True)
            mm.then_inc(m1b)
            nc.tensor.wait_ge(sa2, i + 1)
            nc.tensor.wait_ge(r2b, i)
            for j in range(4):
                mm = nc.tensor.matmul(
                    ps2[:, 512 * j:512 * (j + 1)], lhsT=st2,
                    rhs=S[:, 512 * j:512 * (j + 1)],
                    start=True, stop=True)
            mm.then_inc(m2a)
            nc.tensor.wait_ge(r2a, i + 1)
            for j in range(4):
                mm = nc.tensor.matmul(
                    ps2[:, 512 * j:512 * (j + 1)], lhsT=st2,
                    rhs=S[:, HW_ + 512 * j:HW_ + 512 * (j + 1)],
                    start=True, stop=True)
            mm.then_inc(m2b)

        # ---- DVE loop: 4 row-max reductions ----
        with nc.vector.Fori(lo, hi, 1) as i:
            nc.vector.wait_ge(m1a, i + 1)
            nc.vector.tensor_reduce(
                cm[:, ds(i, 1)], ps1, axis=Ax.X, op=Alu.max).then_inc(r1a)
            nc.vector.wait_ge(m1b, i + 1)
            nc.vector.tensor_reduce(
                cm[:, ds(i + 2 * NBR, 1)], ps1, axis=Ax.X,
                op=Alu.max).then_inc(r1b)
            nc.vector.wait_ge(m2a, i + 1)
            nc.vector.tensor_reduce(
                cm[:, ds(i + NBR, 1)], ps2, axis=Ax.X,
                op=Alu.max).then_inc(r2a)
            nc.vector.wait_ge(m2b, i + 1)
            nc.vector.tensor_reduce(
                cm[:, ds(i + 3 * NBR, 1)], ps2, axis=Ax.X,
                op=Alu.max).then_inc(r2b)

    # ---- tail on DVE ----
    # same-queue waits: no-ops at runtime, explicit sync edges for the
    # race detector against the loop's dynamic-AP reduce writes
    nc.vector.wait_ge(r1a, NBR)
    nc.vector.wait_ge(r1b, NBR)
    nc.vector.wait_ge(r2a, NBR)
    nc.vector.wait_ge(r2b, NBR)
    nc.vector.tensor_max(cm[:, 0:2 * NBR], cm[:, 0:2 * NBR],
                         cm[:, 2 * NBR:4 * NBR]).then_inc(ttail)
    nc.vector.wait_ge(ttail, 1)
    nc.vector.tensor_scalar(
        out=neg, in0=cm[:, 0:2 * NBR], scalar1=-1.0, scalar2=0.0,
        op0=Alu.mult, op1=Alu.max).then_inc(ttail)
    nc.vector.wait_ge(ttail, 2)
    nc.vector.tensor_reduce(
        outsb, neg, axis=Ax.X, op=Alu.add).then_inc(stail)

    # ---- SP: output DMA ----
    nc.sync.wait_ge(stail, 1)
    nc.sync.dma_start(outd, outsb).then_inc(sdone, 16)
    nc.sync.wait_ge(sdone, 16)

    nc.compile()
    _strip_preamble(nc)
    return nc


def _strip_preamble(nc):
    """Drop the dead constructor preamble from the entry block: the four
    const-tensor memsets (nothing in this kernel references the const
    APs) and the initial all-engine barrier (drains + event sems) that
    only ordered them. Everything from the input DMA onward is kept."""
    f = list(nc.m.functions)[0]
    bb = list(f.blocks)[0]
    insts = list(bb.instructions)
    first_dma = next(i for i, x in enumerate(insts)
                     if type(x).__name__ == "InstDMACopy")
    keep = []
    for k, x in enumerate(insts):
        nm = type(x).__name__
        if k < first_dma and nm in ("InstMemset", "InstDrain",
                                    "InstEventSemaphore"):
            continue
        keep.append(x)
    bb.instructions = keep


def _get_program():
    global _PROG
    if _PROG is None:
        _PROG = _build_program()
    return _PROG


def _bf16_round(v: np.ndarray) -> np.ndarray:
    """Round f32 -> nearest-even bf16, returned as f32 with low bits zero."""
    u = np.ascontiguousarray(v, dtype=np.float32).view(np.uint32)
    r = ((u >> 16) & 1) + np.uint32(0x7FFF)
    return ((u + r) & np.uint32(0xFFFF0000)).view(np.float32)


# (i, j) split-product pairs kept: i + j <= 2
_PAIRS = [(0, 0), (0, 1), (0, 2), (1, 0), (1, 1), (2, 0)]


def _split3(v: np.ndarray):
    v = v.astype(np.float32)
    s0 = _bf16_round(v)
    s1 = _bf16_round(v - s0)
    s2 = _bf16_round(v - s0 - s1)
    return s0, s1, s2


def _prep_sides(x: np.ndarray, y: np.ndarray):
    """Build augmented [KA, N] / [KA, M] bf16 matrices (as uint16 bits).

    sum_k A[k,n] * Bm[k,m] = 2<x_n, y_m> - |x_n|^2 - |y_m|^2 = -d2[n,m]
    """
    xs = _split3(x)                      # x ~ xs0+xs1+xs2
    cs = _split3(2.0 * y)                # 2y ~ cs0+cs1+cs2
    nx = _split3((x.astype(np.float64) ** 2).sum(-1).astype(np.float32))
    nyn = _split3(-(y.astype(np.float64) ** 2).sum(-1).astype(np.float32))

    A = np.empty((KA, x.shape[0]), np.float32)
    Bm = np.empty((KA, y.shape[0]), np.float32)
    r = 0
    for d in range(D):
        for (ii, jj) in _PAIRS:
            A[r] = xs[ii][:, d]
            Bm[r] = cs[jj][:, d]
            r += 1
    for k in range(3):
        A[r] = nx[k]
        Bm[r] = -1.0
        r += 1
    for k in range(3):
        A[r] = 1.0
        Bm[r] = nyn[k]
        r += 1
    assert r == KA
    Au = (np.ascontiguousarray(A).view(np.uint32) >> 16).astype(np.uint16)
    Bu = (np.ascontiguousarray(Bm).view(np.uint32) >> 16).astype(np.uint16)
    return Au, Bu


def _prep_s(x: np.ndarray, y: np.ndarray) -> np.ndarray:
    """Augmented matrix pair packed as one [KA, N+M] u16 array."""
    Au, Bu = _prep_sides(x, y)
    return np.concatenate([Au, Bu], axis=1)


def kernel(xyz1: np.ndarray, xyz2: np.ndarray) -> np.ndarray:
    from concourse.bass_utils import run_bass_kernel_spmd

    xyz1 = np.asarray(xyz1, dtype=np.float32)
    xyz2 = np.asarray(xyz2, dtype=np.float32)
    assert xyz1.shape == (B, N, D) and xyz2.shape == (B, M, D)

    nc = _get_program()
    in_maps = []
    for b in range(NCORES):
        in_maps.append({"s": _prep_s(xyz1[b], xyz2[b])})
    res = run_bass_kernel_spmd(nc, in_maps, list(range(NCORES))).results
    total = 0.0
    for r in res:
        total += float(r["out"].astype(np.float64).sum())
    # mean(dist1) + mean(dist2) = (sum dist1 + sum dist2) / (B*N)  (N == M)
    return np.float32(total / (B * N))
